# revision 5
# baseline (speedup 1.0000x reference)
"""BitLinear int2 (ternary-weight) GEMM on 8 NeuronCores.

out[8192, 16384] = (x[8192, 4096] @ w_q[16384, 4096].T) * gamma, fp16 I/O,
fp32 accumulation.

Strategy: tensor-parallel over out_features — each core gets a 2048-row
shard of w_q, x is replicated. Host transposes both operands so the
contraction dim (in_features) lands on SBUF partitions with plain DMAs.
The whole 16MB transposed weight shard stays resident in SBUF; x streams
through in 256-token superblocks; K=4096 accumulates in PSUM across 32
matmuls of [128x128] @ [128x512]. gamma is baked into the PSUM->SBUF
copy as an immediate scale on the scalar engine.
"""

import sys

import numpy as np

for _p in ("/opt/trn_rl_repo", "/root/.axon_site/_ro/trn_rl_repo"):
    if _p not in sys.path:
        sys.path.append(_p)

N_CORES = 8
N_TOKENS = 8192
IN_FEATURES = 4096
OUT_FEATURES = 16384
O_SHARD = OUT_FEATURES // N_CORES  # 2048

P = 128          # partitions / matmul contraction tile
FREE = 512       # matmul moving free dim (one PSUM bank of fp32)
SB = 256         # tokens per x superblock (2 t-tiles)


def _build(gamma: float, T: int = N_TOKENS, K: int = IN_FEATURES, O: int = O_SHARD,
           sb: int = SB):
    import concourse.mybir as mybir
    from concourse import bacc
    from concourse.tile import TileContext

    fp16 = mybir.dt.float16
    fp32 = mybir.dt.float32

    KT = K // P        # 32 k-tiles
    NB = O // FREE     # 4 o-blocks per core
    TT = sb // P       # t-tiles per superblock
    NSB = T // sb      # superblocks

    nc = bacc.Bacc("TRN2", target_bir_lowering=False, debug=False,
                   num_devices=N_CORES)
    xT_d = nc.dram_tensor("xT", (K, T), fp16, kind="ExternalInput")
    wT_d = nc.dram_tensor("wT", (K, O), fp16, kind="ExternalInput")
    out_d = nc.dram_tensor("out", (T, O), fp16, kind="ExternalOutput")

    with TileContext(nc) as tc:
        with tc.tile_pool(name="wpool", bufs=1) as wpool, \
             tc.tile_pool(name="xpool", bufs=2) as xpool, \
             tc.tile_pool(name="opool", bufs=3) as opool, \
             tc.tile_pool(name="psum", bufs=8, space="PSUM") as psum_pool:

            # Resident transposed weights: slab k holds wT[k*128:(k+1)*128, :]
            wt = wpool.tile([P, KT, O], fp16)
            for k in range(KT):
                nc.sync.dma_start(out=wt[:, k, :], in_=wT_d[k * P:(k + 1) * P, :])

            for s in range(NSB):
                t0 = s * sb
                xt = xpool.tile([P, KT, sb], fp16, tag="xt")
                for k in range(KT):
                    nc.sync.dma_start(
                        out=xt[:, k, :], in_=xT_d[k * P:(k + 1) * P, t0:t0 + sb])

                for j in range(TT):
                    ot = opool.tile([P, O], fp16, tag="ot")
                    psums = [psum_pool.tile([P, FREE], fp32, tag="ps",
                                            name=f"ps_{s}_{j}_{ob}")
                             for ob in range(NB)]
                    for k in range(KT):
                        lhsT = xt[:, k, j * P:(j + 1) * P]
                        for ob in range(NB):
                            nc.tensor.matmul(
                                psums[ob],
                                lhsT=lhsT,
                                rhs=wt[:, k, ob * FREE:(ob + 1) * FREE],
                                start=(k == 0),
                                stop=(k == KT - 1),
                            )
                    for ob in range(NB):
                        nc.scalar.mul(
                            out=ot[:, ob * FREE:(ob + 1) * FREE],
                            in_=psums[ob],
                            mul=gamma,
                        )
                    row = t0 + j * P
                    nc.sync.dma_start(out=out_d[row:row + P, :], in_=ot)

    nc.compile()
    return nc


def _run(inputs, trace=False):
    from concourse.bass_utils import run_bass_kernel_spmd

    x = np.asarray(inputs["x"])
    w = np.asarray(inputs["w_q"])
    gamma = float(np.asarray(inputs["gamma"]).astype(np.float32))

    xT = np.ascontiguousarray(x.T)
    nc = _build(gamma)
    in_maps = []
    for c in range(N_CORES):
        wT_c = np.ascontiguousarray(w[c * O_SHARD:(c + 1) * O_SHARD, :].T)
        in_maps.append({"xT": xT, "wT": wT_c})

    res = run_bass_kernel_spmd(nc, in_maps, core_ids=list(range(N_CORES)),
                               trace=trace)
    out = np.concatenate(
        [np.asarray(res.results[c]["out"]) for c in range(N_CORES)], axis=1)
    return out.astype(np.float16, copy=False), res


def kernel(**inputs) -> np.ndarray:
    out, _ = _run(inputs, trace=False)
    return out


# revision 6
# speedup vs baseline: 1.0151x; 1.0151x over previous
"""BitLinear int2 (ternary-weight) GEMM on 8 NeuronCores.

out[8192, 16384] = (x[8192, 4096] @ w_q[16384, 4096].T) * gamma, fp16 I/O,
fp32 accumulation.

Strategy: tensor-parallel over out_features — each core gets a 2048-row
shard of w_q, x is replicated. Host transposes both operands so the
contraction dim (in_features) lands on SBUF partitions with plain DMAs.
The whole 16MB transposed weight shard stays resident in SBUF; x streams
through in 256-token superblocks; K=4096 accumulates in PSUM across 32
matmuls of [128x128] @ [128x512]. gamma is baked into the PSUM->SBUF
copy as an immediate scale on the scalar engine.
"""

import sys

import numpy as np

for _p in ("/opt/trn_rl_repo", "/root/.axon_site/_ro/trn_rl_repo"):
    if _p not in sys.path:
        sys.path.append(_p)

N_CORES = 8
N_TOKENS = 8192
IN_FEATURES = 4096
OUT_FEATURES = 16384
O_SHARD = OUT_FEATURES // N_CORES  # 2048

P = 128          # partitions / matmul contraction tile
FREE = 512       # matmul moving free dim (one PSUM bank of fp32)
SB = 256         # tokens per x superblock (2 t-tiles)


def _build(gamma: float, T: int = N_TOKENS, K: int = IN_FEATURES, O: int = O_SHARD,
           sb: int = SB):
    import concourse.mybir as mybir
    from concourse import bacc
    from concourse.tile import TileContext

    fp16 = mybir.dt.float16
    fp32 = mybir.dt.float32

    KT = K // P        # 32 k-tiles
    NB = O // FREE     # 4 o-blocks per core
    TT = sb // P       # t-tiles per superblock
    NSB = T // sb      # superblocks

    nc = bacc.Bacc("TRN2", target_bir_lowering=False, debug=False,
                   num_devices=N_CORES)
    xT_d = nc.dram_tensor("xT", (K, T), fp16, kind="ExternalInput")
    wT_d = nc.dram_tensor("wT", (K, O), fp16, kind="ExternalInput")
    out_d = nc.dram_tensor("out", (T, O), fp16, kind="ExternalOutput")

    with TileContext(nc) as tc:
        with tc.tile_pool(name="wpool", bufs=1) as wpool, \
             tc.tile_pool(name="xpool", bufs=2) as xpool, \
             tc.tile_pool(name="opool", bufs=3) as opool, \
             tc.tile_pool(name="psum", bufs=8, space="PSUM") as psum_pool:

            # First x superblock lands before the weight stream so the PE
            # can start as soon as weight slab 0 arrives.
            xts = {}
            xts[0] = xpool.tile([P, KT, sb], fp16, tag="xt", name="xt_0")
            for k in range(KT):
                nc.sync.dma_start(
                    out=xts[0][:, k, :], in_=xT_d[k * P:(k + 1) * P, 0:sb])

            # Resident transposed weights, one tile per k-slab so matmul
            # dependencies are per-slab: the k-loop of the first superblock
            # paces along the arriving weight stream instead of waiting for
            # the full 16MB.
            wts = []
            for k in range(KT):
                wk = wpool.tile([P, O], fp16, name=f"wk_{k}")
                nc.sync.dma_start(out=wk[:], in_=wT_d[k * P:(k + 1) * P, :])
                wts.append(wk)

            def copyback(ot, psums, row):
                for ob in range(NB):
                    nc.scalar.mul(
                        out=ot[:, ob * FREE:(ob + 1) * FREE],
                        in_=psums[ob],
                        mul=gamma,
                    )
                nc.sync.dma_start(out=out_d[row:row + P, :], in_=ot)

            for s in range(NSB):
                t0 = s * sb
                if s not in xts:
                    xts[s] = xpool.tile([P, KT, sb], fp16, tag="xt",
                                        name=f"xt_{s}")
                    for k in range(KT):
                        nc.sync.dma_start(
                            out=xts[s][:, k, :],
                            in_=xT_d[k * P:(k + 1) * P, t0:t0 + sb])
                xt = xts[s]

                if s == 0:
                    # Interleave both t-tiles k-outer: 8 matmuls per weight
                    # slab keeps the PE ahead of the DMA stream during the
                    # resident-weight fill. Uses all 8 PSUM banks.
                    ots = [opool.tile([P, O], fp16, tag="ot", name=f"ot_{s}_{j}")
                           for j in range(TT)]
                    psums = [[psum_pool.tile([P, FREE], fp32, tag="ps",
                                             name=f"ps_{s}_{j}_{ob}")
                              for ob in range(NB)] for j in range(TT)]
                    for k in range(KT):
                        for j in range(TT):
                            lhsT = xt[:, k, j * P:(j + 1) * P]
                            for ob in range(NB):
                                nc.tensor.matmul(
                                    psums[j][ob],
                                    lhsT=lhsT,
                                    rhs=wts[k][:, ob * FREE:(ob + 1) * FREE],
                                    start=(k == 0),
                                    stop=(k == KT - 1),
                                )
                    for j in range(TT):
                        copyback(ots[j], psums[j], t0 + j * P)
                else:
                    for j in range(TT):
                        ot = opool.tile([P, O], fp16, tag="ot",
                                        name=f"ot_{s}_{j}")
                        psums = [psum_pool.tile([P, FREE], fp32, tag="ps",
                                                name=f"ps_{s}_{j}_{ob}")
                                 for ob in range(NB)]
                        for k in range(KT):
                            lhsT = xt[:, k, j * P:(j + 1) * P]
                            for ob in range(NB):
                                nc.tensor.matmul(
                                    psums[ob],
                                    lhsT=lhsT,
                                    rhs=wts[k][:, ob * FREE:(ob + 1) * FREE],
                                    start=(k == 0),
                                    stop=(k == KT - 1),
                                )
                        copyback(ot, psums, t0 + j * P)

    nc.compile()
    return nc


def _run(inputs, trace=False):
    from concourse.bass_utils import run_bass_kernel_spmd

    x = np.asarray(inputs["x"])
    w = np.asarray(inputs["w_q"])
    gamma = float(np.asarray(inputs["gamma"]).astype(np.float32))

    xT = np.ascontiguousarray(x.T)
    nc = _build(gamma)
    in_maps = []
    for c in range(N_CORES):
        wT_c = np.ascontiguousarray(w[c * O_SHARD:(c + 1) * O_SHARD, :].T)
        in_maps.append({"xT": xT, "wT": wT_c})

    res = run_bass_kernel_spmd(nc, in_maps, core_ids=list(range(N_CORES)),
                               trace=trace)
    out = np.concatenate(
        [np.asarray(res.results[c]["out"]) for c in range(N_CORES)], axis=1)
    return out.astype(np.float16, copy=False), res


def kernel(**inputs) -> np.ndarray:
    out, _ = _run(inputs, trace=False)
    return out


# revision 14
# speedup vs baseline: 1.0223x; 1.0071x over previous
"""BitLinear int2 (ternary-weight) GEMM on 8 NeuronCores.

out[8192, 16384] = (x[8192, 4096] @ w_q[16384, 4096].T) * gamma, fp16 I/O,
fp32 accumulation.

Strategy: tensor-parallel over out_features — each core gets a 2048-row
shard of w_q, x is replicated. Host transposes both operands so the
contraction dim (in_features) lands on SBUF partitions with plain DMAs.
The whole 16MB transposed weight shard stays resident in SBUF; x streams
through in 256-token superblocks; K=4096 accumulates in PSUM across 32
matmuls of [128x128] @ [128x512]. gamma is baked into the PSUM->SBUF
copy as an immediate scale on the scalar engine.
"""

import sys

import numpy as np

for _p in ("/opt/trn_rl_repo", "/root/.axon_site/_ro/trn_rl_repo"):
    if _p not in sys.path:
        sys.path.append(_p)

N_CORES = 8
N_TOKENS = 8192
IN_FEATURES = 4096
OUT_FEATURES = 16384
O_SHARD = OUT_FEATURES // N_CORES  # 2048

P = 128          # partitions / matmul contraction tile
FREE = 512       # matmul moving free dim (one PSUM bank of fp32)
SB = 256         # tokens per x superblock (2 t-tiles)


def _build(gamma: float, T: int = N_TOKENS, K: int = IN_FEATURES, O: int = O_SHARD,
           sb: int = SB):
    import concourse.mybir as mybir
    from concourse import bacc
    from concourse.tile import TileContext

    fp16 = mybir.dt.float16
    fp32 = mybir.dt.float32

    KT = K // P        # 32 k-tiles
    NB = O // FREE     # 4 o-blocks per core
    TT = sb // P       # t-tiles per superblock
    NSB = T // sb      # superblocks

    nc = bacc.Bacc("TRN2", target_bir_lowering=False, debug=False,
                   num_devices=N_CORES)
    # x is host-packed to [128, NSB, KT, sb]: per partition, one superblock's
    # slabs are contiguous (16KB runs -> line-rate DMA descriptors).
    xQ_d = nc.dram_tensor("xQ", (P, NSB, KT, sb), fp16, kind="ExternalInput")
    wT_d = nc.dram_tensor("wT", (K, O), fp16, kind="ExternalInput")
    out_d = nc.dram_tensor("out", (T, O), fp16, kind="ExternalOutput")

    XCH = 4 if KT % 4 == 0 else 1  # x DMA chunks per superblock
    KC = KT // XCH                 # k-slabs per chunk

    with TileContext(nc) as tc:
        with tc.tile_pool(name="wpool", bufs=1) as wpool, \
             tc.tile_pool(name="xpool", bufs=2) as xpool, \
             tc.tile_pool(name="opool", bufs=3) as opool, \
             tc.tile_pool(name="psum", bufs=8, space="PSUM") as psum_pool:

            # x loads ride the ACT HWDGE ring; weights + outputs ride the SP
            # ring, so weight slab 0 is not queued behind x transfers.
            def load_x(xt, s):
                for c in range(XCH):
                    nc.scalar.dma_start(
                        out=xt[:, c * KC:(c + 1) * KC, :],
                        in_=xQ_d[:, s, c * KC:(c + 1) * KC, :])

            xts = {}
            xts[0] = xpool.tile([P, KT, sb], fp16, tag="xt", name="xt_0")
            load_x(xts[0], 0)

            # Resident transposed weights, one tile per k-slab so matmul
            # dependencies are per-slab: the k-loop of the first superblock
            # paces along the arriving weight stream instead of waiting for
            # the full 16MB.
            wts = []
            for k in range(KT):
                wk = wpool.tile([P, O], fp16, name=f"wk_{k}")
                nc.sync.dma_start(out=wk[:], in_=wT_d[k * P:(k + 1) * P, :])
                wts.append(wk)

            def copyback(ot, psums, row, split=False):
                for ob in range(NB):
                    nc.scalar.mul(
                        out=ot[:, ob * FREE:(ob + 1) * FREE],
                        in_=psums[ob],
                        mul=gamma,
                    )
                    if split:
                        nc.sync.dma_start(
                            out=out_d[row:row + P, ob * FREE:(ob + 1) * FREE],
                            in_=ot[:, ob * FREE:(ob + 1) * FREE])
                if not split:
                    nc.sync.dma_start(out=out_d[row:row + P, :], in_=ot)

            for s in range(NSB):
                t0 = s * sb
                if s not in xts:
                    xts[s] = xpool.tile([P, KT, sb], fp16, tag="xt",
                                        name=f"xt_{s}")
                    load_x(xts[s], s)
                xt = xts[s]

                if s == 0:
                    # Interleave both t-tiles k-outer: 8 matmuls per weight
                    # slab keeps the PE ahead of the DMA stream during the
                    # resident-weight fill. Uses all 8 PSUM banks.
                    ots = [opool.tile([P, O], fp16, tag="ot", name=f"ot_{s}_{j}")
                           for j in range(TT)]
                    psums = [[psum_pool.tile([P, FREE], fp32, tag="ps",
                                             name=f"ps_{s}_{j}_{ob}")
                              for ob in range(NB)] for j in range(TT)]
                    for k in range(KT):
                        for j in range(TT):
                            lhsT = xt[:, k, j * P:(j + 1) * P]
                            for ob in range(NB):
                                nc.tensor.matmul(
                                    psums[j][ob],
                                    lhsT=lhsT,
                                    rhs=wts[k][:, ob * FREE:(ob + 1) * FREE],
                                    start=(k == 0),
                                    stop=(k == KT - 1),
                                )
                    for j in range(TT):
                        copyback(ots[j], psums[j], t0 + j * P)
                else:
                    for j in range(TT):
                        ot = opool.tile([P, O], fp16, tag="ot",
                                        name=f"ot_{s}_{j}")
                        psums = [psum_pool.tile([P, FREE], fp32, tag="ps",
                                                name=f"ps_{s}_{j}_{ob}")
                                 for ob in range(NB)]
                        for k in range(KT):
                            lhsT = xt[:, k, j * P:(j + 1) * P]
                            for ob in range(NB):
                                nc.tensor.matmul(
                                    psums[ob],
                                    lhsT=lhsT,
                                    rhs=wts[k][:, ob * FREE:(ob + 1) * FREE],
                                    start=(k == 0),
                                    stop=(k == KT - 1),
                                )
                        copyback(ot, psums, t0 + j * P,
                                 split=(s == NSB - 1 and j == TT - 1))

    nc.compile()
    return nc


def _run(inputs, trace=False):
    import os

    from concourse.bass_utils import run_bass_kernel_spmd

    if not trace:
        # A stray BASS_TRACE would route run_bass_kernel_spmd into the NTFF
        # hook import, which this container lacks.
        os.environ["BASS_NEVER_TRACE"] = "1"
    else:
        os.environ.pop("BASS_NEVER_TRACE", None)

    x = np.asarray(inputs["x"])
    w = np.asarray(inputs["w_q"])
    gamma = float(np.asarray(inputs["gamma"]).astype(np.float32))

    # Pack x to [128, NSB, KT, sb]: xQ[p, s, k, t] = x[s*sb + t, k*128 + p]
    KT, NSB = IN_FEATURES // P, N_TOKENS // SB
    xQ = np.ascontiguousarray(
        x.T.reshape(KT, P, NSB, SB).transpose(1, 2, 0, 3))
    nc = _build(gamma)
    in_maps = []
    for c in range(N_CORES):
        wT_c = np.ascontiguousarray(w[c * O_SHARD:(c + 1) * O_SHARD, :].T)
        in_maps.append({"xQ": xQ, "wT": wT_c})

    res = run_bass_kernel_spmd(nc, in_maps, core_ids=list(range(N_CORES)),
                               trace=trace)
    out = np.concatenate(
        [np.asarray(res.results[c]["out"]) for c in range(N_CORES)], axis=1)
    return out.astype(np.float16, copy=False), res


def kernel(**inputs) -> np.ndarray:
    out, _ = _run(inputs, trace=False)
    return out


# revision 19
# speedup vs baseline: 1.0263x; 1.0039x over previous
"""BitLinear int2 (ternary-weight) GEMM on 8 NeuronCores.

out[8192, 16384] = (x[8192, 4096] @ w_q[16384, 4096].T) * gamma, fp16 I/O,
fp32 accumulation.

Strategy: tensor-parallel over out_features — each core gets a 2048-row
shard of w_q, x is replicated. Host transposes both operands so the
contraction dim (in_features) lands on SBUF partitions with plain DMAs.
The whole 16MB transposed weight shard stays resident in SBUF; x streams
through in 256-token superblocks; K=4096 accumulates in PSUM across 32
matmuls of [128x128] @ [128x512]. gamma is baked into the PSUM->SBUF
copy as an immediate scale on the scalar engine.
"""

import sys

import numpy as np

for _p in ("/opt/trn_rl_repo", "/root/.axon_site/_ro/trn_rl_repo"):
    if _p not in sys.path:
        sys.path.append(_p)

N_CORES = 8
N_TOKENS = 8192
IN_FEATURES = 4096
OUT_FEATURES = 16384
O_SHARD = OUT_FEATURES // N_CORES  # 2048

P = 128          # partitions / matmul contraction tile
FREE = 512       # matmul moving free dim (one PSUM bank of fp32)
SB = 256         # tokens per x superblock (2 t-tiles)


def _build(gamma: float, T: int = N_TOKENS, K: int = IN_FEATURES, O: int = O_SHARD,
           sb: int = SB):
    import concourse.mybir as mybir
    from concourse import bacc
    from concourse.tile import TileContext

    fp16 = mybir.dt.float16
    fp32 = mybir.dt.float32

    KT = K // P        # 32 k-tiles
    NB = O // FREE     # 4 o-blocks per core
    TT = sb // P       # t-tiles per superblock
    NSB = T // sb      # superblocks

    nc = bacc.Bacc("TRN2", target_bir_lowering=False, debug=False,
                   num_devices=N_CORES)
    # x is host-packed to [128, NSB, KT, sb]: per partition, one superblock's
    # slabs are contiguous (16KB runs -> line-rate DMA descriptors).
    xQ_d = nc.dram_tensor("xQ", (P, NSB, KT, sb), fp16, kind="ExternalInput")
    wT_d = nc.dram_tensor("wT", (K, O), fp16, kind="ExternalInput")
    out_d = nc.dram_tensor("out", (T, O), fp16, kind="ExternalOutput")

    XCH = 4 if KT % 4 == 0 else 1  # x DMA chunks per superblock
    KC = KT // XCH                 # k-slabs per chunk

    with TileContext(nc) as tc:
        with tc.tile_pool(name="wpool", bufs=1) as wpool, \
             tc.tile_pool(name="xpool", bufs=2) as xpool, \
             tc.tile_pool(name="opool", bufs=3) as opool, \
             tc.tile_pool(name="psum", bufs=8, space="PSUM") as psum_pool:

            # x loads ride the ACT HWDGE ring; weights + outputs ride the SP
            # ring, so weight slab 0 is not queued behind x transfers.
            # Superblock 1 instead queues on the SP ring behind the weight
            # stream: it isn't needed until ~60us and must not steal HBM
            # bandwidth from the resident-weight fill.
            def load_x(xt, s, eng=None):
                eng = eng or nc.scalar
                for c in range(XCH):
                    eng.dma_start(
                        out=xt[:, c * KC:(c + 1) * KC, :],
                        in_=xQ_d[:, s, c * KC:(c + 1) * KC, :])

            xts = {}
            xts[0] = xpool.tile([P, KT, sb], fp16, tag="xt", name="xt_0")
            load_x(xts[0], 0)

            # Resident transposed weights, one tile per (k-slab, o-half) so
            # matmul dependencies are fine-grained: the k-loop of the first
            # superblock paces along the arriving weight stream instead of
            # waiting for the full 16MB.
            OH = O // 2
            wts = {}
            for k in range(KT):
                for h in range(2):
                    wk = wpool.tile([P, OH], fp16, name=f"wk_{k}_{h}")
                    nc.sync.dma_start(
                        out=wk[:],
                        in_=wT_d[k * P:(k + 1) * P, h * OH:(h + 1) * OH])
                    wts[(k, h)] = wk

            def w_rhs(k, ob):
                off = ob * FREE
                return wts[(k, off // OH)][:, off % OH:off % OH + FREE]

            def copyback(ot, psums, row, split=False):
                for ob in range(NB):
                    nc.scalar.mul(
                        out=ot[:, ob * FREE:(ob + 1) * FREE],
                        in_=psums[ob],
                        mul=gamma,
                    )
                    if split:
                        nc.sync.dma_start(
                            out=out_d[row:row + P, ob * FREE:(ob + 1) * FREE],
                            in_=ot[:, ob * FREE:(ob + 1) * FREE])
                if not split:
                    nc.sync.dma_start(out=out_d[row:row + P, :], in_=ot)

            for s in range(NSB):
                t0 = s * sb
                if s not in xts:
                    xts[s] = xpool.tile([P, KT, sb], fp16, tag="xt",
                                        name=f"xt_{s}")
                    load_x(xts[s], s, eng=nc.sync if s == 1 else None)
                xt = xts[s]

                if s == 0:
                    # Interleave both t-tiles k-outer: 8 matmuls per weight
                    # slab keeps the PE ahead of the DMA stream during the
                    # resident-weight fill. Uses all 8 PSUM banks.
                    ots = [opool.tile([P, O], fp16, tag="ot", name=f"ot_{s}_{j}")
                           for j in range(TT)]
                    psums = [[psum_pool.tile([P, FREE], fp32, tag="ps",
                                             name=f"ps_{s}_{j}_{ob}")
                              for ob in range(NB)] for j in range(TT)]
                    for k in range(KT):
                        for j in range(TT):
                            lhsT = xt[:, k, j * P:(j + 1) * P]
                            for ob in range(NB):
                                nc.tensor.matmul(
                                    psums[j][ob],
                                    lhsT=lhsT,
                                    rhs=w_rhs(k, ob),
                                    start=(k == 0),
                                    stop=(k == KT - 1),
                                )
                    for j in range(TT):
                        copyback(ots[j], psums[j], t0 + j * P)
                else:
                    for j in range(TT):
                        ot = opool.tile([P, O], fp16, tag="ot",
                                        name=f"ot_{s}_{j}")
                        row = t0 + j * P
                        last = (s == NSB - 1 and j == TT - 1)
                        if last:
                            # o-block-major: each block's copy + store
                            # overlaps the next block's accumulation, so
                            # only one block's epilogue trails the PE.
                            for ob in range(NB):
                                ps = psum_pool.tile(
                                    [P, FREE], fp32, tag="ps",
                                    name=f"ps_{s}_{j}_{ob}")
                                for k in range(KT):
                                    nc.tensor.matmul(
                                        ps,
                                        lhsT=xt[:, k, j * P:(j + 1) * P],
                                        rhs=w_rhs(k, ob),
                                        start=(k == 0),
                                        stop=(k == KT - 1),
                                    )
                                nc.scalar.mul(
                                    out=ot[:, ob * FREE:(ob + 1) * FREE],
                                    in_=ps,
                                    mul=gamma,
                                )
                                nc.sync.dma_start(
                                    out=out_d[row:row + P,
                                              ob * FREE:(ob + 1) * FREE],
                                    in_=ot[:, ob * FREE:(ob + 1) * FREE])
                            continue
                        psums = [psum_pool.tile([P, FREE], fp32, tag="ps",
                                                name=f"ps_{s}_{j}_{ob}")
                                 for ob in range(NB)]
                        for k in range(KT):
                            lhsT = xt[:, k, j * P:(j + 1) * P]
                            for ob in range(NB):
                                nc.tensor.matmul(
                                    psums[ob],
                                    lhsT=lhsT,
                                    rhs=w_rhs(k, ob),
                                    start=(k == 0),
                                    stop=(k == KT - 1),
                                )
                        copyback(ot, psums, row)

    nc.compile()
    return nc


def _run(inputs, trace=False):
    import os

    from concourse.bass_utils import run_bass_kernel_spmd

    if not trace:
        # A stray BASS_TRACE would route run_bass_kernel_spmd into the NTFF
        # hook import, which this container lacks.
        os.environ["BASS_NEVER_TRACE"] = "1"
    else:
        os.environ.pop("BASS_NEVER_TRACE", None)

    x = np.asarray(inputs["x"])
    w = np.asarray(inputs["w_q"])
    gamma = float(np.asarray(inputs["gamma"]).astype(np.float32))

    # Pack x to [128, NSB, KT, sb]: xQ[p, s, k, t] = x[s*sb + t, k*128 + p]
    KT, NSB = IN_FEATURES // P, N_TOKENS // SB
    xQ = np.ascontiguousarray(
        x.T.reshape(KT, P, NSB, SB).transpose(1, 2, 0, 3))
    nc = _build(gamma)
    in_maps = []
    for c in range(N_CORES):
        wT_c = np.ascontiguousarray(w[c * O_SHARD:(c + 1) * O_SHARD, :].T)
        in_maps.append({"xQ": xQ, "wT": wT_c})

    res = run_bass_kernel_spmd(nc, in_maps, core_ids=list(range(N_CORES)),
                               trace=trace)
    out = np.concatenate(
        [np.asarray(res.results[c]["out"]) for c in range(N_CORES)], axis=1)
    return out.astype(np.float16, copy=False), res


def kernel(**inputs) -> np.ndarray:
    out, _ = _run(inputs, trace=False)
    return out


# revision 20
# speedup vs baseline: 1.0270x; 1.0007x over previous
"""BitLinear int2 (ternary-weight) GEMM on 8 NeuronCores.

out[8192, 16384] = (x[8192, 4096] @ w_q[16384, 4096].T) * gamma, fp16 I/O,
fp32 accumulation.

Strategy: tensor-parallel over out_features — each core gets a 2048-row
shard of w_q, x is replicated. Host transposes both operands so the
contraction dim (in_features) lands on SBUF partitions with plain DMAs.
The whole 16MB transposed weight shard stays resident in SBUF; x streams
through in 256-token superblocks; K=4096 accumulates in PSUM across 32
matmuls of [128x128] @ [128x512]. gamma is baked into the PSUM->SBUF
copy as an immediate scale on the scalar engine.
"""

import sys

import numpy as np

for _p in ("/opt/trn_rl_repo", "/root/.axon_site/_ro/trn_rl_repo"):
    if _p not in sys.path:
        sys.path.append(_p)

N_CORES = 8
N_TOKENS = 8192
IN_FEATURES = 4096
OUT_FEATURES = 16384
O_SHARD = OUT_FEATURES // N_CORES  # 2048

P = 128          # partitions / matmul contraction tile
FREE = 512       # matmul moving free dim (one PSUM bank of fp32)
SB = 256         # tokens per x superblock (2 t-tiles)


def _build(gamma: float, T: int = N_TOKENS, K: int = IN_FEATURES, O: int = O_SHARD,
           sb: int = SB):
    import concourse.mybir as mybir
    from concourse import bacc
    from concourse.tile import TileContext

    fp16 = mybir.dt.float16
    fp32 = mybir.dt.float32

    KT = K // P        # 32 k-tiles
    NB = O // FREE     # 4 o-blocks per core
    TT = sb // P       # t-tiles per superblock
    NSB = T // sb      # superblocks

    nc = bacc.Bacc("TRN2", target_bir_lowering=False, debug=False,
                   num_devices=N_CORES)
    # x is host-packed to [128, NSB, KT, sb]: per partition, one superblock's
    # slabs are contiguous (16KB runs -> line-rate DMA descriptors).
    xQ_d = nc.dram_tensor("xQ", (P, NSB, KT, sb), fp16, kind="ExternalInput")
    wT_d = nc.dram_tensor("wT", (K, O), fp16, kind="ExternalInput")
    out_d = nc.dram_tensor("out", (T, O), fp16, kind="ExternalOutput")

    XCH = 8 if KT % 8 == 0 else 1  # x DMA chunks per superblock
    KC = KT // XCH                 # k-slabs per chunk

    with TileContext(nc) as tc:
        with tc.tile_pool(name="wpool", bufs=1) as wpool, \
             tc.tile_pool(name="xpool", bufs=2) as xpool, \
             tc.tile_pool(name="opool", bufs=3) as opool, \
             tc.tile_pool(name="psum", bufs=8, space="PSUM") as psum_pool:

            # x loads ride the ACT HWDGE ring; weights + outputs ride the SP
            # ring, so weight slab 0 is not queued behind x transfers.
            # Superblock 1 instead queues on the SP ring behind the weight
            # stream: it isn't needed until ~60us and must not steal HBM
            # bandwidth from the resident-weight fill.
            def load_x(xt, s, eng=None):
                eng = eng or nc.scalar
                for c in range(XCH):
                    eng.dma_start(
                        out=xt[:, c * KC:(c + 1) * KC, :],
                        in_=xQ_d[:, s, c * KC:(c + 1) * KC, :])

            xts = {}
            xts[0] = xpool.tile([P, KT, sb], fp16, tag="xt", name="xt_0")
            load_x(xts[0], 0)

            # Resident transposed weights, one tile per (k-slab, o-half) so
            # matmul dependencies are fine-grained: the k-loop of the first
            # superblock paces along the arriving weight stream instead of
            # waiting for the full 16MB.
            OH = O // 2
            wts = {}
            for k in range(KT):
                for h in range(2):
                    wk = wpool.tile([P, OH], fp16, name=f"wk_{k}_{h}")
                    nc.sync.dma_start(
                        out=wk[:],
                        in_=wT_d[k * P:(k + 1) * P, h * OH:(h + 1) * OH])
                    wts[(k, h)] = wk

            def w_rhs(k, ob):
                off = ob * FREE
                return wts[(k, off // OH)][:, off % OH:off % OH + FREE]

            def copyback(ot, psums, row, split=False):
                for ob in range(NB):
                    nc.scalar.mul(
                        out=ot[:, ob * FREE:(ob + 1) * FREE],
                        in_=psums[ob],
                        mul=gamma,
                    )
                    if split:
                        nc.sync.dma_start(
                            out=out_d[row:row + P, ob * FREE:(ob + 1) * FREE],
                            in_=ot[:, ob * FREE:(ob + 1) * FREE])
                if not split:
                    nc.sync.dma_start(out=out_d[row:row + P, :], in_=ot)

            for s in range(NSB):
                t0 = s * sb
                if s not in xts:
                    xts[s] = xpool.tile([P, KT, sb], fp16, tag="xt",
                                        name=f"xt_{s}")
                    load_x(xts[s], s, eng=nc.sync if s == 1 else None)
                xt = xts[s]

                if s == 0:
                    # Interleave both t-tiles k-outer: 8 matmuls per weight
                    # slab keeps the PE ahead of the DMA stream during the
                    # resident-weight fill. Uses all 8 PSUM banks.
                    ots = [opool.tile([P, O], fp16, tag="ot", name=f"ot_{s}_{j}")
                           for j in range(TT)]
                    psums = [[psum_pool.tile([P, FREE], fp32, tag="ps",
                                             name=f"ps_{s}_{j}_{ob}")
                              for ob in range(NB)] for j in range(TT)]
                    for k in range(KT):
                        for j in range(TT):
                            lhsT = xt[:, k, j * P:(j + 1) * P]
                            for ob in range(NB):
                                nc.tensor.matmul(
                                    psums[j][ob],
                                    lhsT=lhsT,
                                    rhs=w_rhs(k, ob),
                                    start=(k == 0),
                                    stop=(k == KT - 1),
                                )
                    for j in range(TT):
                        copyback(ots[j], psums[j], t0 + j * P)
                else:
                    for j in range(TT):
                        ot = opool.tile([P, O], fp16, tag="ot",
                                        name=f"ot_{s}_{j}")
                        row = t0 + j * P
                        last = (s == NSB - 1 and j == TT - 1)
                        if last:
                            # o-block-major: each block's copy + store
                            # overlaps the next block's accumulation, so
                            # only one block's epilogue trails the PE.
                            for ob in range(NB):
                                ps = psum_pool.tile(
                                    [P, FREE], fp32, tag="ps",
                                    name=f"ps_{s}_{j}_{ob}")
                                for k in range(KT):
                                    nc.tensor.matmul(
                                        ps,
                                        lhsT=xt[:, k, j * P:(j + 1) * P],
                                        rhs=w_rhs(k, ob),
                                        start=(k == 0),
                                        stop=(k == KT - 1),
                                    )
                                nc.scalar.mul(
                                    out=ot[:, ob * FREE:(ob + 1) * FREE],
                                    in_=ps,
                                    mul=gamma,
                                )
                                nc.sync.dma_start(
                                    out=out_d[row:row + P,
                                              ob * FREE:(ob + 1) * FREE],
                                    in_=ot[:, ob * FREE:(ob + 1) * FREE])
                            continue
                        psums = [psum_pool.tile([P, FREE], fp32, tag="ps",
                                                name=f"ps_{s}_{j}_{ob}")
                                 for ob in range(NB)]
                        for k in range(KT):
                            lhsT = xt[:, k, j * P:(j + 1) * P]
                            for ob in range(NB):
                                nc.tensor.matmul(
                                    psums[ob],
                                    lhsT=lhsT,
                                    rhs=w_rhs(k, ob),
                                    start=(k == 0),
                                    stop=(k == KT - 1),
                                )
                        copyback(ot, psums, row)

    nc.compile()
    return nc


def _run(inputs, trace=False):
    import os

    from concourse.bass_utils import run_bass_kernel_spmd

    if not trace:
        # A stray BASS_TRACE would route run_bass_kernel_spmd into the NTFF
        # hook import, which this container lacks.
        os.environ["BASS_NEVER_TRACE"] = "1"
    else:
        os.environ.pop("BASS_NEVER_TRACE", None)

    x = np.asarray(inputs["x"])
    w = np.asarray(inputs["w_q"])
    gamma = float(np.asarray(inputs["gamma"]).astype(np.float32))

    # Pack x to [128, NSB, KT, sb]: xQ[p, s, k, t] = x[s*sb + t, k*128 + p]
    KT, NSB = IN_FEATURES // P, N_TOKENS // SB
    xQ = np.ascontiguousarray(
        x.T.reshape(KT, P, NSB, SB).transpose(1, 2, 0, 3))
    nc = _build(gamma)
    in_maps = []
    for c in range(N_CORES):
        wT_c = np.ascontiguousarray(w[c * O_SHARD:(c + 1) * O_SHARD, :].T)
        in_maps.append({"xQ": xQ, "wT": wT_c})

    res = run_bass_kernel_spmd(nc, in_maps, core_ids=list(range(N_CORES)),
                               trace=trace)
    out = np.concatenate(
        [np.asarray(res.results[c]["out"]) for c in range(N_CORES)], axis=1)
    return out.astype(np.float16, copy=False), res


def kernel(**inputs) -> np.ndarray:
    out, _ = _run(inputs, trace=False)
    return out
